# revision 14
# baseline (speedup 1.0000x reference)
"""Trainium2 Bass kernel for nn_DigitConvolutionalModel (3x3 conv + 3-layer MLP).

Math: out = relu(relu(conv3x3(x) @ W1 + b1) @ W2 + b2) @ W3 + b3.

The 3x3 valid conv is linear, so on host we fold it into the first FC:
  conv_flat = x @ A  with A [784, 676] (9 shifted diagonals of conv_w), so
  h1 = relu(x @ W1eff + b1)  with  W1eff = A @ W1 : [784, 256].
K is zero-padded 784 -> 896 = 7*128 so every K-tile is a full 128 partitions.

Sharding: pure data parallel over the batch across 8 cores (2048 rows each).
Each core runs a feature-major 3-layer MLP (activations stored transposed so
every matmul uses the weights as stored, with zero on-device transposes):
  h1T = relu(W1eff.T @ xT + b1)   [256, 2048]
  h2T = relu(W2.T   @ h1T + b2)   [256, 2048]
  oT  =      W3.T   @ h2T + b3    [10, 2048]

Matmuls run in fp16 (full-rate PE streaming + FWL weight loads) with fp32
PSUM accumulation; biases and the output stay fp32.

DMA discipline: each dma_start costs ~600ns of sequencer time and transfers
on one HWDGE ring serialize, so loads are packed on host into contiguous
SBUF-layout blocks and split across BOTH HWDGE rings (SP + ACT) in
consumption order; output stores ride SWDGE (GpSimd) so they never block
loads. A few warm-up matmuls on a zeroed scratch tile run during the DMA
fill so the PE HAM clock-gate is already released when real work arrives.
"""

import numpy as np

import concourse.bacc as bacc
import concourse.bass as bass
import concourse.mybir as mybir
import concourse.tile as tile
from concourse.bass_utils import run_bass_kernel_spmd

N_CORES = 8
B = 16384
B_LOC = B // N_CORES  # 2048 batch rows per core
NCH = 512  # batch chunk per matmul (fp32 PSUM bank = 512 floats)
NCHUNKS = B_LOC // NCH
KIN = 784  # folded input features (28*28)
KPAD = 896  # zero-padded to 7 full 128-row K-tiles
NK1 = KPAD // 128
H = 256
NOUT = 10
NWARM = 4  # PE warm-up matmuls during the DMA fill

F32 = mybir.dt.float32
F16 = mybir.dt.float16
AF = mybir.ActivationFunctionType
ALU = mybir.AluOpType


def build_nc() -> bass.Bass:
    nc = bacc.Bacc(
        "TRN2", target_bir_lowering=False, debug=False, num_devices=N_CORES
    )
    # Host-packed inputs (exact SBUF destination layouts):
    #   xP[ci][k][p][n] = x_shard[ci*NCH+n, k*128+p]
    #   w1P[m][p][k*128+c] = W1eff[k*128+p, m*128+c]
    #   w2P[p][k*H+c] = W2[k*128+p, c];  w3P[p][k*NOUT+c] = W3[k*128+p, c]
    #   bias cols: 0-1 = b1(m), 2-3 = b2(m), 4 = b3 (first 10 rows)
    xP = nc.dram_tensor("xP", [NCHUNKS, NK1, 128, NCH], F16, kind="ExternalInput")
    w1 = nc.dram_tensor("w1", [2, 128, NK1 * 128], F16, kind="ExternalInput")
    w2 = nc.dram_tensor("w2", [128, 2 * H], F16, kind="ExternalInput")
    w3 = nc.dram_tensor("w3", [128, 2 * NOUT], F16, kind="ExternalInput")
    bias = nc.dram_tensor("bias", [128, 5], F32, kind="ExternalInput")
    outT = nc.dram_tensor("outT", [NOUT, B_LOC], F32, kind="ExternalOutput")

    with tile.TileContext(nc) as tc:
        with (
            tc.tile_pool(name="wgt", bufs=1) as wp,
            tc.tile_pool(name="xin", bufs=3) as xp,
            tc.tile_pool(name="act", bufs=2) as hp,
            tc.tile_pool(name="osb", bufs=2) as op,
            tc.tile_pool(name="ps1", bufs=2, space="PSUM") as pp1,
            tc.tile_pool(name="ps2", bufs=1, space="PSUM") as pp2,
            tc.tile_pool(name="ps3", bufs=1, space="PSUM") as pp3,
            tc.tile_pool(name="psw", bufs=1, space="PSUM") as ppw,
        ):
            # PE warm-up: matmuls on a zeroed scratch tile, no DMA deps.
            warm = wp.tile([128, NCH], F16, name="warm")
            nc.gpsimd.memset(warm[:], 0.0)
            psw = ppw.tile([128, NCH], F32, name="psw")
            for _ in range(NWARM):
                nc.tensor.matmul(psw[:], warm[:, 0:128], warm[:], start=True, stop=True)

            # Loads, split across both HWDGE rings in consumption order.
            w1t = []
            for m in range(2):
                t = wp.tile([128, NK1 * 128], F16, name=f"w1_{m}")
                eng = nc.sync if m == 0 else nc.scalar
                eng.dma_start(out=t[:], in_=w1[m])
                w1t.append(t)

            def load_x_chunk(ci):
                xts = []
                for k in range(NK1):
                    t = xp.tile([128, NCH], F16, name="xt", tag=f"x_{k}")
                    eng = nc.sync if k % 2 == 0 else nc.scalar
                    eng.dma_start(out=t[:], in_=xP[ci, k])
                    xts.append(t)
                return xts

            x_next = load_x_chunk(0)

            w2s = wp.tile([128, 2 * H], F16, name="w2s")
            nc.scalar.dma_start(out=w2s[:], in_=w2[:, :])
            w3s = wp.tile([128, 2 * NOUT], F16, name="w3s")
            nc.scalar.dma_start(out=w3s[:], in_=w3[:, :])
            bs = wp.tile([128, 5], F32, name="bs")
            nc.scalar.dma_start(out=bs[:], in_=bias[:, :])

            # Per-engine bias staging (consumer then depends on its own
            # engine in program order instead of an extra DMA semaphore).
            ba = wp.tile([128, 5], F32, name="ba")  # ACT's copy
            nc.scalar.activation(ba[:], bs[:], AF.Copy)
            bv = wp.tile([128, 5], F32, name="bv")  # DVE's copy
            nc.vector.tensor_copy(bv[:], bs[:])
            b1a = [ba[:, 0:1], ba[:, 1:2]]
            b2a = [ba[:, 2:3], ba[:, 3:4]]
            b1v = [bv[:, 0:1], bv[:, 1:2]]
            b2v = [bv[:, 2:3], bv[:, 3:4]]
            b3v = bv[0:NOUT, 4:5]

            # ---- batch-chunk pipeline (x prefetched one chunk ahead) ----
            for ci in range(NCHUNKS):
                n0 = ci * NCH
                xc = x_next
                if ci + 1 < NCHUNKS:
                    x_next = load_x_chunk(ci + 1)

                # layer 1: h1T = relu(W1eff.T @ xT + b1)
                h1 = []
                for m in range(2):
                    ps = pp1.tile([128, NCH], F32, name="ps1", tag=f"ps1_{m}")
                    for k in range(NK1):
                        nc.tensor.matmul(
                            ps[:],
                            w1t[m][:, k * 128 : (k + 1) * 128],
                            xc[k][:],
                            start=(k == 0),
                            stop=(k == NK1 - 1),
                        )
                    h = hp.tile([128, NCH], F16, name="h1", tag=f"h1_{m}")
                    if m == 0:
                        nc.scalar.activation(h[:], ps[:], AF.Relu, bias=b1a[m])
                    else:
                        nc.vector.tensor_scalar(
                            h[:], ps[:], b1v[m], 0.0, ALU.add, ALU.max
                        )
                    h1.append(h)

                # layer 2: h2T = relu(W2.T @ h1T + b2)
                h2 = []
                for m in range(2):
                    ps = pp2.tile([128, NCH], F32, name="ps2", tag=f"ps2_{m}")
                    for k in range(2):
                        nc.tensor.matmul(
                            ps[:],
                            w2s[:, k * H + m * 128 : k * H + (m + 1) * 128],
                            h1[k][:],
                            start=(k == 0),
                            stop=(k == 1),
                        )
                    h = hp.tile([128, NCH], F16, name="h2", tag=f"h2_{m}")
                    if m == 0:
                        nc.scalar.activation(h[:], ps[:], AF.Relu, bias=b2a[m])
                    else:
                        nc.vector.tensor_scalar(
                            h[:], ps[:], b2v[m], 0.0, ALU.add, ALU.max
                        )
                    h2.append(h)

                # layer 3: oT = W3.T @ h2T + b3
                ps = pp3.tile([NOUT, NCH], F32, name="ps3", tag="ps3")
                for k in range(2):
                    nc.tensor.matmul(
                        ps[:],
                        w3s[:, k * NOUT : (k + 1) * NOUT],
                        h2[k][:],
                        start=(k == 0),
                        stop=(k == 1),
                    )
                ob = op.tile([NOUT, NCH], F32, name="ob", tag="ob")
                nc.vector.tensor_scalar(ob[:], ps[:], b3v, None, ALU.add)
                # stores ride SWDGE so they never block HWDGE loads
                nc.gpsimd.dma_start(out=outT[:, n0 : n0 + NCH], in_=ob[:])

    nc.compile()
    return nc


def _fold_conv_into_w1(conv_w: np.ndarray, W1: np.ndarray) -> np.ndarray:
    """W1eff[784, 256] such that x @ W1eff == conv_flat(x, conv_w) @ W1."""
    W1v = W1.astype(np.float64).reshape(26, 26, W1.shape[1])
    cw = conv_w.astype(np.float64)
    acc = np.zeros((28, 28, W1.shape[1]), np.float64)
    for di in range(3):
        for dj in range(3):
            acc[di : di + 26, dj : dj + 26, :] += cw[di, dj] * W1v
    return acc.reshape(KIN, W1.shape[1]).astype(np.float32)


def _pack_kmajor(w: np.ndarray, kpad: int) -> np.ndarray:
    """[K, C] -> [128, (K/128)*C] with row-block k at column block k."""
    k, c = w.shape
    wp = np.zeros((kpad, c), w.dtype)
    wp[:k] = w
    return np.ascontiguousarray(
        wp.reshape(kpad // 128, 128, c).transpose(1, 0, 2).reshape(128, -1)
    )


def _run(inputs: dict, trace: bool = False, tmpdir: str | None = None):
    x = np.asarray(inputs["x"], dtype=np.float32)
    w1e = _fold_conv_into_w1(
        np.asarray(inputs["conv_w"]), np.asarray(inputs["W1"])
    ).astype(np.float16)
    # w1P[m] = k-major pack of W1eff[:, m*128:(m+1)*128]
    w1P = np.stack(
        [_pack_kmajor(w1e[:, m * 128 : (m + 1) * 128], KPAD) for m in range(2)]
    )
    w2P = _pack_kmajor(np.asarray(inputs["W2"], np.float16), H)
    w3P = _pack_kmajor(np.asarray(inputs["W3"], np.float16), H)
    bias = np.zeros((128, 5), np.float32)
    bias[:, 0:2] = np.asarray(inputs["b1"], np.float32).reshape(2, 128).T
    bias[:, 2:4] = np.asarray(inputs["b2"], np.float32).reshape(2, 128).T
    bias[:NOUT, 4] = np.asarray(inputs["b3"], np.float32)

    nc = build_nc()
    in_maps = []
    for c in range(N_CORES):
        xs = np.zeros((B_LOC, KPAD), np.float16)
        xs[:, :KIN] = x[c * B_LOC : (c + 1) * B_LOC]
        # [ci, n, k, p] -> [ci, k, p, n]
        xPc = np.ascontiguousarray(
            xs.reshape(NCHUNKS, NCH, NK1, 128).transpose(0, 2, 3, 1)
        )
        in_maps.append({"xP": xPc, "w1": w1P, "w2": w2P, "w3": w3P, "bias": bias})

    res = run_bass_kernel_spmd(
        nc, in_maps, list(range(N_CORES)), trace=trace, tmpdir=tmpdir
    )
    out = np.concatenate([r["outT"].T for r in res.results], axis=0)
    return np.ascontiguousarray(out.astype(np.float32)), res


def kernel(**inputs) -> np.ndarray:
    out, _ = _run(inputs, trace=False)
    return out


# revision 15
# speedup vs baseline: 1.0310x; 1.0310x over previous
"""Trainium2 Bass kernel for nn_DigitConvolutionalModel (3x3 conv + 3-layer MLP).

Math: out = relu(relu(conv3x3(x) @ W1 + b1) @ W2 + b2) @ W3 + b3.

The 3x3 valid conv is linear, so on host we fold it into the first FC:
  conv_flat = x @ A  with A [784, 676] (9 shifted diagonals of conv_w), so
  h1 = relu(x @ W1eff + b1)  with  W1eff = A @ W1 : [784, 256].
K is zero-padded 784 -> 896 = 7*128 so every K-tile is a full 128 partitions.

Sharding: pure data parallel over the batch across 8 cores (2048 rows each).
Each core runs a feature-major 3-layer MLP (activations stored transposed so
every matmul uses the weights as stored, with zero on-device transposes):
  h1T = relu(W1eff.T @ xT + b1)   [256, 2048]
  h2T = relu(W2.T   @ h1T + b2)   [256, 2048]
  oT  =      W3.T   @ h2T + b3    [10, 2048]

Matmuls run in fp16 (full-rate PE streaming + FWL weight loads) with fp32
PSUM accumulation; biases and the output stay fp32.

DMA discipline: each dma_start costs ~600ns of sequencer time and transfers
on one HWDGE ring serialize, so loads are packed on host into contiguous
SBUF-layout blocks and split across BOTH HWDGE rings (SP + ACT) in
consumption order; output stores ride SWDGE (GpSimd) so they never block
loads. A few warm-up matmuls on a zeroed scratch tile run during the DMA
fill so the PE HAM clock-gate is already released when real work arrives.
"""

import numpy as np

import concourse.bacc as bacc
import concourse.bass as bass
import concourse.mybir as mybir
import concourse.tile as tile
from concourse.bass_utils import run_bass_kernel_spmd

N_CORES = 8
B = 16384
B_LOC = B // N_CORES  # 2048 batch rows per core
NCH = 512  # batch chunk per matmul (fp32 PSUM bank = 512 floats)
NCHUNKS = B_LOC // NCH
KIN = 784  # folded input features (28*28)
KPAD = 896  # zero-padded to 7 full 128-row K-tiles
NK1 = KPAD // 128
H = 256
NOUT = 10
NWARM = 5  # PE warm-up matmuls during the DMA fill
NKA = 4  # k-tiles 0..3 ride the SP ring, 4..6 the ACT ring

F32 = mybir.dt.float32
F16 = mybir.dt.float16
AF = mybir.ActivationFunctionType
ALU = mybir.AluOpType


def build_nc() -> bass.Bass:
    nc = bacc.Bacc(
        "TRN2", target_bir_lowering=False, debug=False, num_devices=N_CORES
    )
    # Host-packed inputs (exact SBUF destination layouts):
    #   xP[ci][k][p][n] = x_shard[ci*NCH+n, k*128+p]
    #   w1P[m][p][k*128+c] = W1eff[k*128+p, m*128+c]
    #   w2P[p][k*H+c] = W2[k*128+p, c];  w3P[p][k*NOUT+c] = W3[k*128+p, c]
    #   bias cols: 0-1 = b1(m), 2-3 = b2(m), 4 = b3 (first 10 rows)
    xA = nc.dram_tensor("xA", [NCHUNKS, 128, NKA * NCH], F16, kind="ExternalInput")
    xB = nc.dram_tensor("xB", [NCHUNKS, 128, (NK1 - NKA) * NCH], F16, kind="ExternalInput")
    w1 = nc.dram_tensor("w1", [2, 128, NK1 * 128], F16, kind="ExternalInput")
    w2 = nc.dram_tensor("w2", [128, 2 * H], F16, kind="ExternalInput")
    w3 = nc.dram_tensor("w3", [128, 2 * NOUT], F16, kind="ExternalInput")
    bias = nc.dram_tensor("bias", [128, 5], F32, kind="ExternalInput")
    outT = nc.dram_tensor("outT", [NOUT, B_LOC], F32, kind="ExternalOutput")

    with tile.TileContext(nc) as tc:
        with (
            tc.tile_pool(name="wgt", bufs=1) as wp,
            tc.tile_pool(name="xin", bufs=3) as xp,
            tc.tile_pool(name="act", bufs=2) as hp,
            tc.tile_pool(name="osb", bufs=2) as op,
            tc.tile_pool(name="ps1", bufs=2, space="PSUM") as pp1,
            tc.tile_pool(name="ps2", bufs=1, space="PSUM") as pp2,
            tc.tile_pool(name="ps3", bufs=1, space="PSUM") as pp3,
            tc.tile_pool(name="psw", bufs=1, space="PSUM") as ppw,
        ):
            # PE warm-up: matmuls on a zeroed scratch tile, no DMA deps.
            warm = wp.tile([128, NCH], F16, name="warm")
            nc.gpsimd.memset(warm[:], 0.0)
            psw = ppw.tile([128, NCH], F32, name="psw")
            for _ in range(NWARM):
                nc.tensor.matmul(psw[:], warm[:, 0:128], warm[:], start=True, stop=True)

            # Loads, split across both HWDGE rings in consumption order.
            w1t = []
            for m in range(2):
                t = wp.tile([128, NK1 * 128], F16, name=f"w1_{m}")
                eng = nc.sync if m == 0 else nc.scalar
                eng.dma_start(out=t[:], in_=w1[m])
                w1t.append(t)

            def load_x_chunk(ci):
                ta = xp.tile([128, NKA * NCH], F16, name="xa", tag="xa")
                nc.sync.dma_start(out=ta[:], in_=xA[ci])
                tb = xp.tile([128, (NK1 - NKA) * NCH], F16, name="xb", tag="xb")
                nc.scalar.dma_start(out=tb[:], in_=xB[ci])
                return (ta, tb)

            x_next = load_x_chunk(0)

            w2s = wp.tile([128, 2 * H], F16, name="w2s")
            nc.scalar.dma_start(out=w2s[:], in_=w2[:, :])
            w3s = wp.tile([128, 2 * NOUT], F16, name="w3s")
            nc.scalar.dma_start(out=w3s[:], in_=w3[:, :])
            bs = wp.tile([128, 5], F32, name="bs")
            nc.scalar.dma_start(out=bs[:], in_=bias[:, :])

            # Per-engine bias staging (consumer then depends on its own
            # engine in program order instead of an extra DMA semaphore).
            ba = wp.tile([128, 5], F32, name="ba")  # ACT's copy
            nc.scalar.activation(ba[:], bs[:], AF.Copy)
            bv = wp.tile([128, 5], F32, name="bv")  # DVE's copy
            nc.vector.tensor_copy(bv[:], bs[:])
            b1a = [ba[:, 0:1], ba[:, 1:2]]
            b2a = [ba[:, 2:3], ba[:, 3:4]]
            b1v = [bv[:, 0:1], bv[:, 1:2]]
            b2v = [bv[:, 2:3], bv[:, 3:4]]
            b3v = bv[0:NOUT, 4:5]

            # ---- batch-chunk pipeline (x prefetched one chunk ahead) ----
            for ci in range(NCHUNKS):
                n0 = ci * NCH
                xc = x_next
                if ci + 1 < NCHUNKS:
                    x_next = load_x_chunk(ci + 1)

                # layer 1: h1T = relu(W1eff.T @ xT + b1)
                h1 = []
                for m in range(2):
                    ps = pp1.tile([128, NCH], F32, name="ps1", tag=f"ps1_{m}")
                    for k in range(NK1):
                        src_t = xc[0] if k < NKA else xc[1]
                        kk = k if k < NKA else k - NKA
                        nc.tensor.matmul(
                            ps[:],
                            w1t[m][:, k * 128 : (k + 1) * 128],
                            src_t[:, kk * NCH : (kk + 1) * NCH],
                            start=(k == 0),
                            stop=(k == NK1 - 1),
                        )
                    h = hp.tile([128, NCH], F16, name="h1", tag=f"h1_{m}")
                    if m == 0:
                        nc.scalar.activation(h[:], ps[:], AF.Relu, bias=b1a[m])
                    else:
                        nc.vector.tensor_scalar(
                            h[:], ps[:], b1v[m], 0.0, ALU.add, ALU.max
                        )
                    h1.append(h)

                # layer 2: h2T = relu(W2.T @ h1T + b2)
                h2 = []
                for m in range(2):
                    ps = pp2.tile([128, NCH], F32, name="ps2", tag=f"ps2_{m}")
                    for k in range(2):
                        nc.tensor.matmul(
                            ps[:],
                            w2s[:, k * H + m * 128 : k * H + (m + 1) * 128],
                            h1[k][:],
                            start=(k == 0),
                            stop=(k == 1),
                        )
                    h = hp.tile([128, NCH], F16, name="h2", tag=f"h2_{m}")
                    if m == 0:
                        nc.scalar.activation(h[:], ps[:], AF.Relu, bias=b2a[m])
                    else:
                        nc.vector.tensor_scalar(
                            h[:], ps[:], b2v[m], 0.0, ALU.add, ALU.max
                        )
                    h2.append(h)

                # layer 3: oT = W3.T @ h2T + b3
                ps = pp3.tile([NOUT, NCH], F32, name="ps3", tag="ps3")
                for k in range(2):
                    nc.tensor.matmul(
                        ps[:],
                        w3s[:, k * NOUT : (k + 1) * NOUT],
                        h2[k][:],
                        start=(k == 0),
                        stop=(k == 1),
                    )
                ob = op.tile([NOUT, NCH], F32, name="ob", tag="ob")
                nc.vector.tensor_scalar(ob[:], ps[:], b3v, None, ALU.add)
                # stores ride SWDGE so they never block HWDGE loads
                nc.gpsimd.dma_start(out=outT[:, n0 : n0 + NCH], in_=ob[:])

    nc.compile()
    return nc


def _fold_conv_into_w1(conv_w: np.ndarray, W1: np.ndarray) -> np.ndarray:
    """W1eff[784, 256] such that x @ W1eff == conv_flat(x, conv_w) @ W1."""
    W1v = W1.astype(np.float64).reshape(26, 26, W1.shape[1])
    cw = conv_w.astype(np.float64)
    acc = np.zeros((28, 28, W1.shape[1]), np.float64)
    for di in range(3):
        for dj in range(3):
            acc[di : di + 26, dj : dj + 26, :] += cw[di, dj] * W1v
    return acc.reshape(KIN, W1.shape[1]).astype(np.float32)


def _pack_kmajor(w: np.ndarray, kpad: int) -> np.ndarray:
    """[K, C] -> [128, (K/128)*C] with row-block k at column block k."""
    k, c = w.shape
    wp = np.zeros((kpad, c), w.dtype)
    wp[:k] = w
    return np.ascontiguousarray(
        wp.reshape(kpad // 128, 128, c).transpose(1, 0, 2).reshape(128, -1)
    )


def _run(inputs: dict, trace: bool = False, tmpdir: str | None = None):
    x = np.asarray(inputs["x"], dtype=np.float32)
    w1e = _fold_conv_into_w1(
        np.asarray(inputs["conv_w"]), np.asarray(inputs["W1"])
    ).astype(np.float16)
    # w1P[m] = k-major pack of W1eff[:, m*128:(m+1)*128]
    w1P = np.stack(
        [_pack_kmajor(w1e[:, m * 128 : (m + 1) * 128], KPAD) for m in range(2)]
    )
    w2P = _pack_kmajor(np.asarray(inputs["W2"], np.float16), H)
    w3P = _pack_kmajor(np.asarray(inputs["W3"], np.float16), H)
    bias = np.zeros((128, 5), np.float32)
    bias[:, 0:2] = np.asarray(inputs["b1"], np.float32).reshape(2, 128).T
    bias[:, 2:4] = np.asarray(inputs["b2"], np.float32).reshape(2, 128).T
    bias[:NOUT, 4] = np.asarray(inputs["b3"], np.float32)

    nc = build_nc()
    in_maps = []
    for c in range(N_CORES):
        xs = np.zeros((B_LOC, KPAD), np.float16)
        xs[:, :KIN] = x[c * B_LOC : (c + 1) * B_LOC]
        # [ci, n, k, p] -> [ci, p, k, n], split at k=NKA into two rings
        arr = xs.reshape(NCHUNKS, NCH, NK1, 128)
        xAc = np.ascontiguousarray(
            arr[:, :, :NKA].transpose(0, 3, 2, 1).reshape(NCHUNKS, 128, -1)
        )
        xBc = np.ascontiguousarray(
            arr[:, :, NKA:].transpose(0, 3, 2, 1).reshape(NCHUNKS, 128, -1)
        )
        in_maps.append(
            {"xA": xAc, "xB": xBc, "w1": w1P, "w2": w2P, "w3": w3P, "bias": bias}
        )

    res = run_bass_kernel_spmd(
        nc, in_maps, list(range(N_CORES)), trace=trace, tmpdir=tmpdir
    )
    out = np.concatenate([r["outT"].T for r in res.results], axis=0)
    return np.ascontiguousarray(out.astype(np.float32)), res


def kernel(**inputs) -> np.ndarray:
    out, _ = _run(inputs, trace=False)
    return out
